# revision 5
# baseline (speedup 1.0000x reference)
"""Trainium2 Bass kernel for ContextAwareAttention.

Math (per batch row b):
    pi  = x[b] @ W_in.T + b_in                  # [S, D]
    pc  = context[b] @ W_ctx.T + b_ctx          # [D]
    h   = tanh(pi + pc)                         # [S, D]
    sc  = h @ w_att (+ b_att, dropped: softmax shift-invariant)   # [S]
    w   = softmax(sc)                           # [S]
    out = w @ x[b]                              # [D]

Sharding: data-parallel over batch, 2 batch rows per NeuronCore x 8 cores.

Device layout choices:
  - x is shipped host-side as xT = x[b].T (bf16, [D, S*2] per core) so the
    TensorEngine contraction over d has d on partitions with zero on-chip
    transposes, and pass 2 (the weighted sum over s) runs as a fused
    multiply+reduce on the VectorEngine over the same xT tiles.
  - W_in / W_ctx shipped pre-transposed ([d, e], bf16).
  - All matmul operands bf16, PSUM accumulation f32 (measured end-to-end
    rel err vs f32 reference: ~3e-3).
"""

import os
import numpy as np
import ml_dtypes

BF16 = ml_dtypes.bfloat16

P = 128          # partitions
D = 1024         # hidden dim
S = 4096         # sequence length
B_FULL = 16      # full batch
N_CORES = 8
B_LOC = B_FULL // N_CORES      # batch rows per core (2)
M = B_LOC * S                  # s-rows per core (8192)
DC = D // P                    # d chunks (8)
NH = 2                         # e halves (512 each, one PSUM bank)
HW = D // NH                   # 512

_BUILT = {}


def _build(m_loc=M, b_loc=B_LOC):
    """Build + compile the per-core Bass module. m_loc/b_loc overridable for
    small-scale simulation tests."""
    import concourse.bass as bass  # noqa: F401
    import concourse.tile as tile
    from concourse import mybir, bacc
    import concourse.bass_isa as bass_isa
    from concourse.masks import make_identity
    from contextlib import ExitStack

    dt = mybir.dt
    s_loc = m_loc // b_loc               # sequence per batch row
    nt = m_loc // P                      # s-tiles total
    tpb = s_loc // P                     # s-tiles per batch row
    SLAB = min(1024, m_loc)              # s-columns per pass-1 slab
    n_blk = m_loc // SLAB
    tps = SLAB // P                      # s-tiles per slab

    nc = bacc.Bacc("TRN2", target_bir_lowering=False, debug=False)

    xT = nc.dram_tensor("xT", [D, m_loc], dt.bfloat16, kind="ExternalInput")
    winT = nc.dram_tensor("winT", [D, D], dt.bfloat16, kind="ExternalInput")
    wctxT = nc.dram_tensor("wctxT", [D, D], dt.bfloat16, kind="ExternalInput")
    ctxT = nc.dram_tensor("ctxT", [D, b_loc], dt.bfloat16, kind="ExternalInput")
    bvec = nc.dram_tensor("bvec", [1, D], dt.bfloat16, kind="ExternalInput")
    wrow = nc.dram_tensor("wrow", [1, D], dt.bfloat16, kind="ExternalInput")
    out_t = nc.dram_tensor("out", [b_loc, D], dt.float32, kind="ExternalOutput")

    xT_r = xT[:].rearrange("(c p) s -> c p s", p=P)        # [DC, P, m_loc]
    winT_r = winT[:].rearrange("(c p) e -> c p e", p=P)    # [DC, P, D]
    wctxT_r = wctxT[:].rearrange("(c p) e -> c p e", p=P)
    ctxT_r = ctxT[:].rearrange("(c p) b -> c p b", p=P)
    out_r = out_t[:].rearrange("b (c q) -> b q c", q=P)    # [b_loc, P, DC]

    with tile.TileContext(nc) as tc, ExitStack() as ctx:
        const = ctx.enter_context(tc.tile_pool(name="const", bufs=1))
        wpool = ctx.enter_context(tc.tile_pool(name="wstream", bufs=2))
        xpool = ctx.enter_context(tc.tile_pool(name="xs", bufs=2))
        hpool = ctx.enter_context(tc.tile_pool(name="hs", bufs=3))
        spool = ctx.enter_context(tc.tile_pool(name="small", bufs=2))
        p2pool = ctx.enter_context(tc.tile_pool(name="p2", bufs=1))
        psum = ctx.enter_context(tc.tile_pool(name="psum", bufs=2, space="PSUM"))
        psum1 = ctx.enter_context(tc.tile_pool(name="psum1", bufs=1, space="PSUM"))
        dram = ctx.enter_context(tc.tile_pool(name="dram", bufs=1, space="DRAM"))

        # ---- constants / prologue ----
        win_sb = const.tile([P, DC, D], dt.bfloat16)
        nc.sync.dma_start(out=win_sb, in_=winT_r.rearrange("c p e -> p c e"))
        wbc = const.tile([P, D], dt.bfloat16)
        nc.sync.dma_start(out=wbc, in_=wrow[:].to_broadcast((P, D)))
        bvec_sb = const.tile([1, D], dt.bfloat16)
        nc.sync.dma_start(out=bvec_sb, in_=bvec[:])
        ctx_sb = const.tile([P, DC, b_loc], dt.bfloat16)
        nc.sync.dma_start(out=ctx_sb, in_=ctxT_r.rearrange("c p b -> p c b"))
        ones_b = const.tile([1, b_loc], dt.bfloat16)
        nc.vector.memset(ones_b, 1.0)
        ident = const.tile([P, P], dt.float32)
        make_identity(nc, ident)

        # ---- context projection: pc[b, e] = ctx[b] @ W_ctx.T + (b_in + b_ctx) ----
        pc_ps = []
        for h in range(NH):
            pp = psum1.tile([b_loc, HW], dt.float32, tag=f"pc{h}", name=f"pc{h}")
            for c in range(DC):
                wslab = wpool.tile([P, HW], dt.bfloat16, tag="wctx")
                nc.sync.dma_start(out=wslab, in_=wctxT_r[c][:, h * HW:(h + 1) * HW])
                nc.tensor.matmul(pp, lhsT=ctx_sb[:, c, :], rhs=wslab,
                                 start=(c == 0), stop=False)
            nc.tensor.matmul(pp, lhsT=ones_b, rhs=bvec_sb[:, h * HW:(h + 1) * HW],
                             start=False, stop=True)
            pc_ps.append(pp)
        pc_sb = const.tile([b_loc, D], dt.bfloat16)
        for h in range(NH):
            nc.scalar.copy(pc_sb[:, h * HW:(h + 1) * HW], pc_ps[h])
        pc_dram = dram.tile([b_loc, D], dt.bfloat16)
        nc.sync.dma_start(out=pc_dram, in_=pc_sb)
        bias_full = []
        for b in range(b_loc):
            bf = const.tile([P, D], dt.bfloat16, tag=f"biasf{b}", name=f"biasf{b}")
            nc.sync.dma_start(out=bf, in_=pc_dram[b:b + 1, :].to_broadcast((P, D)))
            bias_full.append(bf)

        scores = const.tile([P, nt], dt.float32)

        # ---- pass 1: scores[s] = w_att . tanh(x[s] @ W_in.T + pc[b]) ----
        for blk in range(n_blk):
            xsl = []
            for c in range(DC):
                t = xpool.tile([P, SLAB], dt.bfloat16, tag=f"x{c}", name=f"xsl{c}")
                nc.sync.dma_start(out=t, in_=xT_r[c][:, blk * SLAB:(blk + 1) * SLAB])
                xsl.append(t)
            for it in range(tps):
                i = blk * tps + it
                b = i // tpb
                h_ps = [psum.tile([P, HW], dt.float32, tag=f"h{h}", name=f"hps{h}") for h in range(NH)]
                for h in range(NH):
                    for c in range(DC):
                        nc.tensor.matmul(
                            h_ps[h],
                            lhsT=xsl[c][:, it * P:(it + 1) * P],
                            rhs=win_sb[:, c, h * HW:(h + 1) * HW],
                            start=(c == 0), stop=(c == DC - 1),
                        )
                hs = hpool.tile([P, D], dt.bfloat16, tag="hadd")
                for h in range(NH):
                    nc.vector.tensor_add(hs[:, h * HW:(h + 1) * HW], h_ps[h],
                                         bias_full[b][:, h * HW:(h + 1) * HW])
                th = hpool.tile([P, D], dt.bfloat16, tag="tanh")
                nc.scalar.activation(th, hs, mybir.ActivationFunctionType.Tanh)
                scr = hpool.tile([P, D], dt.bfloat16, tag="scr")
                nc.vector.tensor_tensor_reduce(
                    out=scr, in0=th, in1=wbc, scale=1.0, scalar=0.0,
                    op0=mybir.AluOpType.mult, op1=mybir.AluOpType.add,
                    accum_out=scores[:, i:i + 1],
                )

        # ---- softmax over each batch row's s-tiles ----
        pn = const.tile([P, nt], dt.float32)
        for b in range(b_loc):
            sl = scores[:, b * tpb:(b + 1) * tpb]
            mx = spool.tile([P, 1], dt.float32, tag="mx")
            nc.vector.tensor_reduce(mx, sl, axis=mybir.AxisListType.X,
                                    op=mybir.AluOpType.max)
            mxr = spool.tile([P, 1], dt.float32, tag="mxr")
            nc.gpsimd.partition_all_reduce(mxr, mx, channels=P,
                                           reduce_op=bass_isa.ReduceOp.max)
            negm = spool.tile([P, 1], dt.float32, tag="negm")
            nc.vector.tensor_scalar_mul(negm, mxr, -1.0)
            pe_ = spool.tile([P, tpb], dt.float32, tag="pexp")
            nc.scalar.activation(pe_, sl, mybir.ActivationFunctionType.Exp,
                                 bias=negm[:, 0:1], scale=1.0)
            zs = spool.tile([P, 1], dt.float32, tag="zs")
            nc.vector.tensor_reduce(zs, pe_, axis=mybir.AxisListType.X,
                                    op=mybir.AluOpType.add)
            zr = spool.tile([P, 1], dt.float32, tag="zr")
            nc.gpsimd.partition_all_reduce(zr, zs, channels=P,
                                           reduce_op=bass_isa.ReduceOp.add)
            rz = spool.tile([P, 1], dt.float32, tag="rz")
            nc.vector.reciprocal(rz, zr)
            nc.vector.tensor_scalar_mul(pn[:, b * tpb:(b + 1) * tpb], pe_, rz[:, 0:1])

        # ---- p to row-major layout: p_dram[i*P + q] = pn[q, i] ----
        pT_ps = psum1.tile([nt, P], dt.float32, tag="pT")
        nc.tensor.transpose(pT_ps, pn, ident)
        pT_sb = spool.tile([nt, P], dt.float32, tag="pTs")
        nc.scalar.copy(pT_sb, pT_ps)
        p_dram = dram.tile([m_loc], dt.float32)
        nc.sync.dma_start(out=p_dram[:].rearrange("(i q) -> i q", q=P), in_=pT_sb)

        # ---- pass 2: out[b, d] = sum_s p[s] * xT[d, s] ----
        for b in range(b_loc):
            pbf = p2pool.tile([P, s_loc], dt.float32, tag="pb")
            nc.sync.dma_start(
                out=pbf,
                in_=p_dram[b * s_loc:(b + 1) * s_loc]
                .unsqueeze(0).to_broadcast((P, s_loc)),
            )
            pbb = p2pool.tile([P, s_loc], dt.bfloat16, tag="pbb")
            nc.vector.tensor_copy(pbb, pbf)
            outsb = spool.tile([P, DC], dt.float32, tag=f"outsb{b}")
            for c in range(DC):
                xs2 = xpool.tile([P, s_loc], dt.bfloat16, tag="xs2")
                nc.sync.dma_start(out=xs2, in_=xT_r[c][:, b * s_loc:(b + 1) * s_loc])
                junk = p2pool.tile([P, s_loc], dt.bfloat16, tag="junk")
                nc.vector.tensor_tensor_reduce(
                    out=junk, in0=xs2, in1=pbb, scale=1.0, scalar=0.0,
                    op0=mybir.AluOpType.mult, op1=mybir.AluOpType.add,
                    accum_out=outsb[:, c:c + 1],
                )
            nc.sync.dma_start(out=out_r[b], in_=outsb)

    nc.compile()
    return nc


def get_nc(m_loc=M, b_loc=B_LOC):
    key = (m_loc, b_loc)
    if key not in _BUILT:
        _BUILT[key] = _build(m_loc, b_loc)
    return _BUILT[key]


def make_in_maps(x, context, W_in, b_in, W_ctx, b_ctx, w_att):
    """Host-side shard + layout prep. All args np.float32 full tensors."""
    winT = np.ascontiguousarray(np.asarray(W_in, np.float32).T).astype(BF16)
    wctxT = np.ascontiguousarray(np.asarray(W_ctx, np.float32).T).astype(BF16)
    bvec = (np.asarray(b_in, np.float32) + np.asarray(b_ctx, np.float32))
    bvec = bvec.reshape(1, D).astype(BF16)
    wrow = np.asarray(w_att, np.float32).reshape(1, D).astype(BF16)
    in_maps = []
    for k in range(N_CORES):
        xs = np.asarray(x[k * B_LOC:(k + 1) * B_LOC], np.float32).reshape(M, D)
        xTk = np.ascontiguousarray(xs.T).astype(BF16)          # [D, M]
        ctxk = np.asarray(context[k * B_LOC:(k + 1) * B_LOC], np.float32)
        ctxTk = np.ascontiguousarray(ctxk.T).astype(BF16)      # [D, B_LOC]
        in_maps.append({
            "xT": xTk, "winT": winT, "wctxT": wctxT,
            "ctxT": ctxTk, "bvec": bvec, "wrow": wrow,
        })
    return in_maps


def kernel(x, context, W_in, b_in, W_ctx, b_ctx, w_att, b_att):
    # b_att shifts every score equally; softmax is shift-invariant, so it
    # has no effect on the output and is intentionally unused.
    from concourse.bass_utils import run_bass_kernel_spmd

    os.environ.setdefault("BASS_NEVER_TRACE", "1")
    nc = get_nc()
    in_maps = make_in_maps(x, context, W_in, b_in, W_ctx, b_ctx, w_att)
    res = run_bass_kernel_spmd(nc, in_maps, core_ids=list(range(N_CORES)))
    outs = [np.asarray(res.results[k]["out"], np.float32) for k in range(N_CORES)]
    return np.concatenate(outs, axis=0)


# revision 6
# speedup vs baseline: 1.0767x; 1.0767x over previous
"""Trainium2 Bass kernel for ContextAwareAttention.

Math (per batch row b):
    pi  = x[b] @ W_in.T + b_in                  # [S, D]
    pc  = context[b] @ W_ctx.T + b_ctx          # [D]
    h   = tanh(pi + pc)                         # [S, D]
    sc  = h @ w_att (+ b_att, dropped: softmax shift-invariant)   # [S]
    w   = softmax(sc)                           # [S]
    out = w @ x[b]                              # [D]

Sharding: data-parallel over batch, 2 batch rows per NeuronCore x 8 cores.

Device layout choices:
  - x is shipped host-side as xT = x[b].T (bf16, [D, 8192] per core) so the
    TensorEngine contraction over d has d on partitions with zero on-chip
    transposes. xT stays fully resident in SBUF (16 MiB) so pass 2 (the
    weighted sum over s, a fused multiply+reduce on the VectorEngine) reads
    the same tiles -- x crosses HBM once.
  - W_in / W_ctx shipped pre-transposed ([d, e], bf16).
  - Per-batch interleave: batch 0's softmax + pass 2 (DVE/DMA only) overlap
    batch 1's pass-1 matmuls.
  - All matmul operands bf16, PSUM accumulation f32 (measured end-to-end
    rel err vs f32 reference: ~3e-3).
"""

import os
import numpy as np
import ml_dtypes

BF16 = ml_dtypes.bfloat16

P = 128          # partitions
D = 1024         # hidden dim
S = 4096         # sequence length
B_FULL = 16      # full batch
N_CORES = 8
B_LOC = B_FULL // N_CORES      # batch rows per core (2)
M = B_LOC * S                  # s-rows per core (8192)
DC = D // P                    # d chunks (8)
NH = 2                         # e halves (512 each, one PSUM bank)
HW = D // NH                   # 512
XBLK = 1024                    # s-columns per resident xT tile

_BUILT = {}


def _build(m_loc=M, b_loc=B_LOC):
    """Build + compile the per-core Bass module. m_loc/b_loc overridable for
    small-scale simulation tests."""
    import concourse.bass as bass  # noqa: F401
    import concourse.tile as tile
    from concourse import mybir, bacc
    import concourse.bass_isa as bass_isa
    from concourse.masks import make_identity
    from contextlib import ExitStack

    dt = mybir.dt
    s_loc = m_loc // b_loc               # sequence per batch row
    nt = m_loc // P                      # s-tiles total
    tpb = s_loc // P                     # s-tiles per batch row
    xblk = min(XBLK, s_loc)              # s-columns per resident xT tile
    n_blk = m_loc // xblk                # xT column blocks
    tpx = xblk // P                      # s-tiles per x block
    bpb = s_loc // xblk                  # x blocks per batch row

    nc = bacc.Bacc("TRN2", target_bir_lowering=False, debug=False)

    xT = nc.dram_tensor("xT", [D, m_loc], dt.bfloat16, kind="ExternalInput")
    winT = nc.dram_tensor("winT", [D, D], dt.bfloat16, kind="ExternalInput")
    wctxT = nc.dram_tensor("wctxT", [D, D], dt.bfloat16, kind="ExternalInput")
    ctxT = nc.dram_tensor("ctxT", [D, b_loc], dt.bfloat16, kind="ExternalInput")
    bvec = nc.dram_tensor("bvec", [1, D], dt.bfloat16, kind="ExternalInput")
    wrow = nc.dram_tensor("wrow", [1, D], dt.bfloat16, kind="ExternalInput")
    out_t = nc.dram_tensor("out", [b_loc, D], dt.float32, kind="ExternalOutput")

    xT_r = xT[:].rearrange("(c p) s -> c p s", p=P)        # [DC, P, m_loc]
    winT_r = winT[:].rearrange("(c p) e -> c p e", p=P)    # [DC, P, D]
    wctxT_r = wctxT[:].rearrange("(c p) e -> c p e", p=P)
    ctxT_r = ctxT[:].rearrange("(c p) b -> c p b", p=P)
    out_r = out_t[:].rearrange("b (c q) -> b q c", q=P)    # [b_loc, P, DC]

    with tile.TileContext(nc) as tc, ExitStack() as ctx:
        const = ctx.enter_context(tc.tile_pool(name="const", bufs=1))
        xres = ctx.enter_context(tc.tile_pool(name="xres", bufs=1))
        wpool = ctx.enter_context(tc.tile_pool(name="wstream", bufs=2))
        hpool = ctx.enter_context(tc.tile_pool(name="hs", bufs=3))
        spool = ctx.enter_context(tc.tile_pool(name="small", bufs=2))
        psum = ctx.enter_context(tc.tile_pool(name="psum", bufs=2, space="PSUM"))
        psum1 = ctx.enter_context(tc.tile_pool(name="psum1", bufs=1, space="PSUM"))
        dram = ctx.enter_context(tc.tile_pool(name="dram", bufs=1, space="DRAM"))

        # ---- constants ----
        win_sb = const.tile([P, DC, D], dt.bfloat16)
        nc.scalar.dma_start(out=win_sb, in_=winT_r.rearrange("c p e -> p c e"))
        wbc = const.tile([P, D], dt.bfloat16)
        nc.gpsimd.dma_start(out=wbc, in_=wrow[:].to_broadcast((P, D)))
        bvec_sb = const.tile([1, D], dt.bfloat16)
        nc.gpsimd.dma_start(out=bvec_sb, in_=bvec[:])
        ctx_sb = const.tile([P, DC, b_loc], dt.bfloat16)
        nc.gpsimd.dma_start(out=ctx_sb, in_=ctxT_r.rearrange("c p b -> p c b"))
        ones_b = const.tile([1, b_loc], dt.bfloat16)
        nc.vector.memset(ones_b, 1.0)
        ident = const.tile([P, P], dt.float32)
        make_identity(nc, ident)

        # ---- resident xT tiles: [DC][n_blk] of [P, xblk], loaded once ----
        xt = [[None] * n_blk for _ in range(DC)]
        for blk in range(n_blk):
            for c in range(DC):
                t = xres.tile([P, xblk], dt.bfloat16, tag=f"xt{c}_{blk}",
                              name=f"xt{c}_{blk}")
                nc.sync.dma_start(out=t, in_=xT_r[c][:, blk * xblk:(blk + 1) * xblk])
                xt[c][blk] = t

        # ---- context projection: pc[b, e] = ctx[b] @ W_ctx.T + (b_in + b_ctx) ----
        pc_ps = []
        for h in range(NH):
            pp = psum1.tile([b_loc, HW], dt.float32, tag=f"pc{h}", name=f"pc{h}")
            for c in range(DC):
                wslab = wpool.tile([P, HW], dt.bfloat16, tag="wctx")
                nc.scalar.dma_start(out=wslab, in_=wctxT_r[c][:, h * HW:(h + 1) * HW])
                nc.tensor.matmul(pp, lhsT=ctx_sb[:, c, :], rhs=wslab,
                                 start=(c == 0), stop=False)
            nc.tensor.matmul(pp, lhsT=ones_b, rhs=bvec_sb[:, h * HW:(h + 1) * HW],
                             start=False, stop=True)
            pc_ps.append(pp)
        pc_sb = const.tile([b_loc, D], dt.bfloat16)
        for h in range(NH):
            nc.scalar.copy(pc_sb[:, h * HW:(h + 1) * HW], pc_ps[h])
        pc_dram = dram.tile([b_loc, D], dt.bfloat16)
        nc.gpsimd.dma_start(out=pc_dram, in_=pc_sb)
        bias_full = []
        for b in range(b_loc):
            bf = const.tile([P, D], dt.bfloat16, tag=f"biasf{b}", name=f"biasf{b}")
            nc.gpsimd.dma_start(out=bf, in_=pc_dram[b:b + 1, :].to_broadcast((P, D)))
            bias_full.append(bf)

        scores = const.tile([P, nt], dt.float32)
        p_dram = dram.tile([m_loc], dt.bfloat16)

        # ---- per batch row: pass 1 -> softmax -> pass 2 ----
        for b in range(b_loc):
            # pass 1: scores[s] = w_att . tanh(x[s] @ W_in.T + pc[b])
            for ib in range(tpb):
                i = b * tpb + ib               # global s-tile
                blk = i // tpx                 # resident x block
                it = i % tpx                   # tile within block
                h_ps = [psum.tile([P, HW], dt.float32, tag=f"h{h}", name=f"hps{h}")
                        for h in range(NH)]
                for h in range(NH):
                    for c in range(DC):
                        nc.tensor.matmul(
                            h_ps[h],
                            lhsT=xt[c][blk][:, it * P:(it + 1) * P],
                            rhs=win_sb[:, c, h * HW:(h + 1) * HW],
                            start=(c == 0), stop=(c == DC - 1),
                        )
                hs = hpool.tile([P, D], dt.bfloat16, tag="hadd")
                for h in range(NH):
                    nc.vector.tensor_add(hs[:, h * HW:(h + 1) * HW], h_ps[h],
                                         bias_full[b][:, h * HW:(h + 1) * HW])
                th = hpool.tile([P, D], dt.bfloat16, tag="tanh")
                nc.scalar.activation(th, hs, mybir.ActivationFunctionType.Tanh)
                scr = hpool.tile([P, D], dt.bfloat16, tag="scr")
                nc.vector.tensor_tensor_reduce(
                    out=scr, in0=th, in1=wbc, scale=1.0, scalar=0.0,
                    op0=mybir.AluOpType.mult, op1=mybir.AluOpType.add,
                    accum_out=scores[:, i:i + 1],
                )

            # softmax over this batch row's tpb score columns
            sl = scores[:, b * tpb:(b + 1) * tpb]
            mx = spool.tile([P, 1], dt.float32, tag="mx")
            nc.vector.tensor_reduce(mx, sl, axis=mybir.AxisListType.X,
                                    op=mybir.AluOpType.max)
            mxr = spool.tile([P, 1], dt.float32, tag="mxr")
            nc.gpsimd.partition_all_reduce(mxr, mx, channels=P,
                                           reduce_op=bass_isa.ReduceOp.max)
            negm = spool.tile([P, 1], dt.float32, tag="negm")
            nc.vector.tensor_scalar_mul(negm, mxr, -1.0)
            pe_ = spool.tile([P, tpb], dt.float32, tag="pexp")
            nc.scalar.activation(pe_, sl, mybir.ActivationFunctionType.Exp,
                                 bias=negm[:, 0:1], scale=1.0)
            zs = spool.tile([P, 1], dt.float32, tag="zs")
            nc.vector.tensor_reduce(zs, pe_, axis=mybir.AxisListType.X,
                                    op=mybir.AluOpType.add)
            zr = spool.tile([P, 1], dt.float32, tag="zr")
            nc.gpsimd.partition_all_reduce(zr, zs, channels=P,
                                           reduce_op=bass_isa.ReduceOp.add)
            rz = spool.tile([P, 1], dt.float32, tag="rz")
            nc.vector.reciprocal(rz, zr)
            pn = spool.tile([P, tpb], dt.float32, tag="pn")
            nc.vector.tensor_scalar_mul(pn, pe_, rz[:, 0:1])

            # p to row-major s order: p_dram[b*s_loc + i*P + q] = pn[q, i]
            pT_ps = psum1.tile([tpb, P], dt.float32, tag="pT", name="pT")
            nc.tensor.transpose(pT_ps, pn, ident)
            pT_sb = spool.tile([tpb, P], dt.bfloat16, tag="pTs")
            nc.scalar.copy(pT_sb, pT_ps)
            nc.gpsimd.dma_start(
                out=p_dram[b * s_loc:(b + 1) * s_loc]
                .rearrange("(i q) -> i q", q=P),
                in_=pT_sb)
            pbb = const.tile([P, s_loc], dt.bfloat16, tag=f"pbb{b}", name=f"pbb{b}")
            nc.gpsimd.dma_start(
                out=pbb,
                in_=p_dram[b * s_loc:(b + 1) * s_loc]
                .unsqueeze(0).to_broadcast((P, s_loc)))

            # pass 2: out[b, c*P + q] = sum_s p[s] * xT[c*P + q, s]
            outsb = spool.tile([P, DC], dt.float32, tag=f"outsb{b}", name=f"outsb{b}")
            for c in range(DC):
                for k in range(bpb):
                    blk = b * bpb + k
                    junk = hpool.tile([P, xblk], dt.bfloat16, tag="junk")
                    nc.vector.tensor_tensor_reduce(
                        out=junk, in0=xt[c][blk],
                        in1=pbb[:, k * xblk:(k + 1) * xblk],
                        scale=1.0,
                        scalar=(0.0 if k == 0 else outsb[:, c:c + 1]),
                        op0=mybir.AluOpType.mult, op1=mybir.AluOpType.add,
                        accum_out=outsb[:, c:c + 1],
                    )
            nc.sync.dma_start(out=out_r[b], in_=outsb)

    nc.compile()
    return nc


def get_nc(m_loc=M, b_loc=B_LOC):
    key = (m_loc, b_loc)
    if key not in _BUILT:
        _BUILT[key] = _build(m_loc, b_loc)
    return _BUILT[key]


def make_in_maps(x, context, W_in, b_in, W_ctx, b_ctx, w_att):
    """Host-side shard + layout prep. All args np.float32 full tensors."""
    winT = np.ascontiguousarray(np.asarray(W_in, np.float32).T).astype(BF16)
    wctxT = np.ascontiguousarray(np.asarray(W_ctx, np.float32).T).astype(BF16)
    bvec = (np.asarray(b_in, np.float32) + np.asarray(b_ctx, np.float32))
    bvec = bvec.reshape(1, D).astype(BF16)
    wrow = np.asarray(w_att, np.float32).reshape(1, D).astype(BF16)
    in_maps = []
    for k in range(N_CORES):
        xs = np.asarray(x[k * B_LOC:(k + 1) * B_LOC], np.float32).reshape(M, D)
        xTk = np.ascontiguousarray(xs.T).astype(BF16)          # [D, M]
        ctxk = np.asarray(context[k * B_LOC:(k + 1) * B_LOC], np.float32)
        ctxTk = np.ascontiguousarray(ctxk.T).astype(BF16)      # [D, B_LOC]
        in_maps.append({
            "xT": xTk, "winT": winT, "wctxT": wctxT,
            "ctxT": ctxTk, "bvec": bvec, "wrow": wrow,
        })
    return in_maps


def kernel(x, context, W_in, b_in, W_ctx, b_ctx, w_att, b_att):
    # b_att shifts every score equally; softmax is shift-invariant, so it
    # has no effect on the output and is intentionally unused.
    from concourse.bass_utils import run_bass_kernel_spmd

    os.environ.setdefault("BASS_NEVER_TRACE", "1")
    nc = get_nc()
    in_maps = make_in_maps(x, context, W_in, b_in, W_ctx, b_ctx, w_att)
    res = run_bass_kernel_spmd(nc, in_maps, core_ids=list(range(N_CORES)))
    outs = [np.asarray(res.results[k]["out"], np.float32) for k in range(N_CORES)]
    return np.concatenate(outs, axis=0)
